# revision 23
# baseline (speedup 1.0000x reference)
"""Trainium2 Bass kernel for nn_Attention_32650341384246.

Full attention layer: qkv proj + per-head RMSNorm(q,k) + RoPE + softmax
attention (non-causal) + out proj.  B=2, S=2048, D=1024, H=16, DH=64.

Sharding: 8 cores; core c handles batch c//4, heads [4*(c%4), 4*(c%4)+4)
(data parallel over batch x tensor parallel over heads).  Each core
computes a partial [S, D] output (its heads @ Wout row-slice); the host
sums the 4 partials per batch and adds the (folded) biases.

Device design (per core):
  - x fed pre-transposed+bf16 as xT [128, 8, 2048]  (p + 128*a = model dim)
  - qkv proj emits qT/kT head-major [128 (2 heads x 64), S] directly
    (lhsT = W slice, rhs = xT slice) and v s-major [s, 4*64].
  - RMSNorm in head-major layout: sum(x^2) over d via ones-block matmul,
    rsqrt directly on ACT (AF.Abs_reciprocal_sqrt; its table set also has
    identity -> exactly 2 ACT table loads in the whole kernel: one in
    phase 1, the exp set in phase 2), partition-broadcast via ones matmul.
  - RoPE as q_rot = cosT*u + sinT'*swap(u); swap = adjacent-partition
    permutation matmul; cos/sin tables host-built bf16 from `pos` with
    q_scale/k_scale folded in; rope DVE muls run bf16 (2x mode where
    both operands are SBUF bf16).
  - scores^T [k, q] bf16 matmuls (K=64, tile_position row groups, qc
    inner so each LDWEIGHTS serves 2 matmuls), PSUM [128, 1024] per
    head.
  - exp split across engines to unblock the PE: head 0 of each pair on
    ACT (table exp, bf16 out), head 1 on DVE via a Schraudolph fast-exp
    (bits16 = round(score*c + d) -> int16 tile bitcast bf16; the
    constant bias of the approximation cancels exactly in softmax).
  - AV via lhsT = [v | ones] bf16 (M=65): row 64 accumulates sumexp.
  - normalize: per-row RECIPROCAL_APPROX_FAST straight off the PSUM
    sumexp rows into 32-aligned partitions of one table, select-matrix
    matmul broadcasts reciprocal rows across partitions.
  - out proj: lhsT = v_mixT bf16, rhs = Wout row-slice bf16, n inner so
    each LDWEIGHTS serves 2 matmuls.
"""
import sys, os

sys.path.insert(0, "/opt/trn_rl_repo")

import numpy as np
from contextlib import ExitStack

import ml_dtypes
import concourse.bass as bass
import concourse.mybir as mybir
import concourse.tile as tile
from concourse import bacc
from concourse import bass_utils

F32 = mybir.dt.float32
F32R = mybir.dt.float32r
BF16 = mybir.dt.bfloat16
I16 = mybir.dt.int16
AF = mybir.ActivationFunctionType
ALU = mybir.AluOpType

B, S, DM, H, DH = 2, 2048, 1024, 16, 64
NC = 8
HPC = H // 4          # 4 heads per core
HD = HPC * DH         # 256
NDT = DM // 128       # 8 model-dim tiles
THETA, EPS = 10000.0, 1e-6
LOG2E = 1.4426950408889634

# kt iterations (of 16) whose pair-head-1 exp runs on DVE (Schraudolph);
# the rest go to ACT.  Tuned so ACT and DVE finish together.
DVE_KTS = frozenset(range(16))

LAST_RESULTS = None   # BassKernelResults of the most recent device run
_CACHED = {}


def build_program(exp_scale: float, shared_tables: bool):
    nc = bacc.Bacc("TRN2", target_bir_lowering=False, debug=False)

    xT_d = nc.dram_tensor("xT", [128, NDT, S], BF16, kind="ExternalInput")
    w_d = nc.dram_tensor("w_all", [128, NDT, 3 * HD], BF16, kind="ExternalInput")
    wout_d = nc.dram_tensor("wout", [128, 2, DM], BF16, kind="ExternalInput")
    bq_d = nc.dram_tensor("bq", [128, 2], F32, kind="ExternalInput")
    bk_d = nc.dram_tensor("bk", [128, 2], F32, kind="ExternalInput")
    cosk_d = nc.dram_tensor("cos_k", [128, S], BF16, kind="ExternalInput")
    sink_d = nc.dram_tensor("sin_k", [128, S], BF16, kind="ExternalInput")
    if not shared_tables:
        cosq_d = nc.dram_tensor("cos_q", [128, S], BF16, kind="ExternalInput")
        sinq_d = nc.dram_tensor("sin_q", [128, S], BF16, kind="ExternalInput")
    P_d = nc.dram_tensor("Pswap", [128, 128], BF16, kind="ExternalInput")
    ob_d = nc.dram_tensor("onesblk", [128, 2], BF16, kind="ExternalInput")
    o2_d = nc.dram_tensor("ones2blk", [2, 128], F32R, kind="ExternalInput")
    sel_d = nc.dram_tensor("sel", [128, 2, 128], F32R, kind="ExternalInput")
    out_d = nc.dram_tensor("outp", [S, DM], BF16, kind="ExternalOutput")

    # Schraudolph fast-exp constants: bits16(x) = x*sc_c + sc_d, int16,
    # bitcast bf16 ~= 2^(x*exp_scale*log2e + const).  The 2^const factor
    # cancels in softmax normalization.
    sc_c = float(exp_scale * 128.0 * LOG2E)
    sc_d = float(127.0 * 128.0 - 6.0)

    with tile.TileContext(nc) as tc, ExitStack() as ctx, \
            nc.allow_low_precision(reason="fp32r/bf16 matmul inputs"):
        singles = ctx.enter_context(tc.tile_pool(name="singles", bufs=1))
        tmp = ctx.enter_context(tc.tile_pool(name="tmp", bufs=2))
        expp = ctx.enter_context(tc.tile_pool(name="expp", bufs=4))
        outp = ctx.enter_context(tc.tile_pool(name="outp", bufs=4))

        # --- first-needed loads up front; per-dt tiles so Tile's
        # per-tile RAW tracking doesn't serialize readers behind all DMAs ---
        w_dt = [singles.tile([128, 3 * HD], BF16, name=f"w{dt}") for dt in range(NDT)]
        x_dt = [singles.tile([128, S], BF16, name=f"x{dt}") for dt in range(NDT)]
        for dt in range(NDT):
            nc.sync.dma_start(out=w_dt[dt], in_=w_d.ap()[:, dt, :])
            nc.sync.dma_start(out=x_dt[dt], in_=xT_d.ap()[:, dt, :])

        wout = singles.tile([128, 2, DM], BF16)
        nc.sync.dma_start(out=wout, in_=wout_d.ap())
        bq = singles.tile([128, 2], F32)
        nc.sync.dma_start(out=bq, in_=bq_d.ap())
        bk = singles.tile([128, 2], F32)
        nc.sync.dma_start(out=bk, in_=bk_d.ap())
        cos_k = singles.tile([128, S], BF16)
        nc.sync.dma_start(out=cos_k, in_=cosk_d.ap())
        sin_k = singles.tile([128, S], BF16)
        nc.sync.dma_start(out=sin_k, in_=sink_d.ap())
        if shared_tables:
            cos_q, sin_q = cos_k, sin_k
        else:
            cos_q = singles.tile([128, S], BF16)
            nc.sync.dma_start(out=cos_q, in_=cosq_d.ap())
            sin_q = singles.tile([128, S], BF16)
            nc.sync.dma_start(out=sin_q, in_=sinq_d.ap())
        Pm = singles.tile([128, 128], BF16)
        nc.sync.dma_start(out=Pm, in_=P_d.ap())
        onesblk = singles.tile([128, 2], BF16)
        nc.sync.dma_start(out=onesblk, in_=ob_d.ap())
        ones2blk = singles.tile([2, 128], F32R)
        nc.sync.dma_start(out=ones2blk, in_=o2_d.ap())
        sel = singles.tile([128, 2, 128], F32R)
        nc.sync.dma_start(out=sel, in_=sel_d.ap())
        eps_t = singles.tile([128, 1], F32)
        nc.vector.memset(eps_t, EPS)
        recip4 = singles.tile([128, 512], F32R)

        qt = [[singles.tile([128, 512], BF16, name=f"qt{t}_{sc}")
               for sc in range(4)] for t in range(2)]
        kt_ = [[singles.tile([128, 512], BF16, name=f"kt{t}_{sc}")
                for sc in range(4)] for t in range(2)]
        # v columns padded to 128 (cols 65..127 zero) so the AV LDWEIGHTS
        # sees a full 128-column bf16 weight and fast-weight-load kicks in
        vhat = [singles.tile([128, 4, HPC, 128], BF16, name=f"vhat{sc}")
                for sc in range(4)]
        for sc in range(4):
            nc.vector.memset(vhat[sc][:, :, :, 64:128], 0.0)
            nc.vector.memset(vhat[sc][:, :, :, 64:65], 1.0)
        vmix = [[singles.tile([128, 1024], BF16, name=f"vmix{t}_{qh}")
                 for qh in range(2)] for t in range(2)]

        # ---------------- phase 1: qkv + rmsnorm + rope ----------------
        # section-major order [k0, q0, k1, q1, v].  ACT uses only
        # Identity + Rsqrt here (one table set); rope DVE muls are bf16.
        with tc.tile_pool(name="ps1", bufs=1, space="PSUM") as ps1:
            # 8 pipeline units (4 sections x 2 s-chunk-pairs), stage-lagged
            # emission: each unit's helper matmuls (pss/pb/psw) enter the PE
            # FIFO 1-3 rounds after the unit's own pq matmuls, so their
            # ACT/DVE producers are always done and the PE never head-stalls.
            units = []
            for which, t, bias, cosT, sinT, dest in (
                    ("k", 0, bk, cos_k, sin_k, kt_),
                    ("q", 0, bq, cos_q, sin_q, qt),
                    ("k", 1, bk, cos_k, sin_k, kt_),
                    ("q", 1, bq, cos_q, sin_q, qt)):
                for scp in range(2):
                    units.append((which, t, bias, cosT, sinT, dest, scp))
            stv = [dict() for _ in units]

            def S0(r):      # PE: 16 pq MMs (dt-outer, shared LDW); ACT: tt
                which, t, bias, cosT, sinT, dest, scp = units[r]
                off = 0 if which == "q" else HD
                pqs = [ps1.tile([128, 512], F32, tag="pq", bufs=4,
                                name=f"pq{which}{t}_{scp * 2 + j}")
                       for j in range(2)]
                for dt in range(NDT):
                    for j in range(2):
                        nc.tensor.matmul(
                            pqs[j][:, :],
                            w_dt[dt][:, off + t * 128: off + (t + 1) * 128],
                            x_dt[dt][:, (scp * 2 + j) * 512:(scp * 2 + j + 1) * 512],
                            start=(dt == 0), stop=(dt == NDT - 1),
                            skip_group_check=True)
                tts = []
                for j in range(2):
                    tt = tmp.tile([128, 512], BF16, tag="tt", bufs=6,
                                  name=f"tt{which}{t}_{scp * 2 + j}")
                    nc.scalar.activation(tt[:, :], pqs[j][:, :], AF.Identity,
                                         bias=bias[:, t:t + 1], scale=1.0)
                    tts.append(tt)
                stv[r]["tts"] = tts

            def S1(r):      # DVE: sq; PE: pss; ACT: rs
                which, t, bias, cosT, sinT, dest, scp = units[r]
                rss = []
                for j in range(2):
                    sq = tmp.tile([128, 512], BF16, tag=f"sq{j}",
                                  name=f"sq{which}{t}_{scp * 2 + j}")
                    nc.vector.tensor_mul(sq[:, :], stv[r]["tts"][j][:, :],
                                         stv[r]["tts"][j][:, :])
                    pss = ps1.tile([2, 512], F32, tag="pss", bufs=2,
                                   name=f"pss{which}{t}_{scp * 2 + j}")
                    nc.tensor.matmul(pss[:, :], onesblk[:, :], sq[:, :],
                                     start=True, stop=True)
                    rs = tmp.tile([2, 512], F32R, tag="rs", bufs=4,
                                  name=f"rs{which}{t}_{scp * 2 + j}")
                    nc.scalar.activation(rs[:, :], pss[:, :],
                                         AF.Abs_reciprocal_sqrt,
                                         bias=eps_t[0:2, :], scale=1.0 / DH)
                    rss.append(rs)
                stv[r]["rss"] = rss

            def S2(r):      # PE: pb; DVE: u, t1
                which, t, bias, cosT, sinT, dest, scp = units[r]
                us, t1s = [], []
                for j in range(2):
                    pb = ps1.tile([128, 512], F32, tag="pb",
                                  name=f"pb{which}{t}_{scp * 2 + j}")
                    nc.tensor.matmul(pb[:, :], ones2blk[:, :],
                                     stv[r]["rss"][j][:, :],
                                     start=True, stop=True)
                    u = tmp.tile([128, 512], BF16, tag="u", bufs=4,
                                 name=f"u{which}{t}_{scp * 2 + j}")
                    nc.vector.tensor_mul(u[:, :], stv[r]["tts"][j][:, :],
                                         pb[:, :])
                    s0 = (scp * 2 + j) * 512
                    t1 = tmp.tile([128, 512], BF16, tag="t1", bufs=4,
                                  name=f"t1{which}{t}_{scp * 2 + j}")
                    nc.vector.tensor_mul(t1[:, :], u[:, :],
                                         cosT[:, s0:s0 + 512])
                    us.append(u)
                    t1s.append(t1)
                stv[r]["us"], stv[r]["t1s"] = us, t1s

            def S3(r):      # PE: psw; DVE: t2, add -> qt/kt dest
                which, t, bias, cosT, sinT, dest, scp = units[r]
                for j in range(2):
                    psw = ps1.tile([128, 512], F32, tag="psw",
                                   name=f"psw{which}{t}_{scp * 2 + j}")
                    nc.tensor.matmul(psw[:, :], Pm[:, :],
                                     stv[r]["us"][j][:, :],
                                     start=True, stop=True)
                    s0 = (scp * 2 + j) * 512
                    t2 = tmp.tile([128, 512], BF16, tag=f"t2{j}",
                                  name=f"t2{which}{t}_{scp * 2 + j}")
                    nc.vector.tensor_mul(t2[:, :], psw[:, :],
                                         sinT[:, s0:s0 + 512])
                    nc.vector.tensor_add(dest[t][scp * 2 + j][:, :],
                                         stv[r]["t1s"][j][:, :], t2[:, :])

            for r in range(len(units) + 3):
                if r < len(units):
                    S0(r)
                if 0 <= r - 1 < len(units):
                    S1(r - 1)
                if 0 <= r - 2 < len(units):
                    S2(r - 2)
                if 0 <= r - 3 < len(units):
                    S3(r - 3)

            # v projection last (Tile spreads its matmuls into earlier gaps);
            # pv shares the pq tag/slots ([128,512] shape, low half used)
            for sc in range(4):
                for st in range(4):
                    pv = ps1.tile([128, 512], F32, tag="pq", bufs=4,
                                  name=f"pv{sc}_{st}")
                    for dt in range(NDT):
                        nc.tensor.matmul(
                            pv[:, 0:HD],
                            x_dt[dt][:, sc * 512 + st * 128: sc * 512 + (st + 1) * 128],
                            w_dt[dt][:, 2 * HD:3 * HD],
                            start=(dt == 0), stop=(dt == NDT - 1))
                    nc.vector.tensor_copy(vhat[sc][:, st, :, 0:64],
                                          pv[:, 0:HD].rearrange("p (h d) -> p h d", h=HPC))

        # ---------------- phase 2: attention ----------------
        from concourse.dve_ops import (RECIP_APPROX_FAST_CONSTS,
                                       RECIPROCAL_APPROX_FAST)
        _c = RECIP_APPROX_FAST_CONSTS
        with tc.tile_pool(name="ps2", bufs=1, space="PSUM") as ps2:
            # HAM heater: ~3.5us of gapless back-to-back matmuls so the PE
            # clock-gate opens (1.2 -> 2.4 GHz) before the attention loop,
            # whose exp-paced micro-gaps never present a full busy window.
            heat = ps2.tile([128, 512], F32, tag="av00", name="heat")
            for i in range(16):
                nc.tensor.matmul(heat[:, :], Pm[:, :], qt[0][0][:, :],
                                 start=True, stop=True, skip_group_check=True)
            for pair in range(2):
                for qh in range(2):
                    q0 = qh * 1024
                    ps_sc = [ps2.tile([128, 1024], F32, tag=f"sc{h}",
                                      name=f"sc{pair}{qh}{h}") for h in range(2)]
                    ps_av = [[ps2.tile([128, 512], F32, tag=f"av{h}{qc}",
                                       name=f"av{pair}{qh}{h}{qc}")
                              for qc in range(2)] for h in range(2)]
                    for kt in range(16):
                        # scores: h outer (LDWEIGHTS shared across the two
                        # qc matmuls)
                        for h in range(2):
                            for qc in range(2):
                                nc.tensor.matmul(
                                    ps_sc[h][:, qc * 512:(qc + 1) * 512],
                                    kt_[pair][kt // 4][h * 64:(h + 1) * 64,
                                                       (kt % 4) * 128:(kt % 4 + 1) * 128],
                                    qt[pair][qh * 2 + qc][h * 64:(h + 1) * 64, :],
                                    start=True, stop=True,
                                    tile_position=(h * 64, 0))
                        # exp: h0 on ACT (table), h1 on DVE (Schraudolph)
                        e0 = expp.tile([128, 1024], BF16, tag="e0",
                                       name=f"e{pair}{qh}0_{kt}")
                        nc.scalar.activation(e0[:, :], ps_sc[0][:, :], AF.Exp,
                                             scale=exp_scale)
                        e1i = expp.tile([128, 1024], I16, tag="e1",
                                        name=f"e{pair}{qh}1_{kt}")
                        nc.vector.tensor_scalar(
                            e1i[:, :], ps_sc[1][:, :], sc_c, sc_d,
                            ALU.mult, ALU.add)
                        e1 = e1i.bitcast(BF16)
                        es = (e0, e1)
                        for h in range(2):
                            head = 2 * pair + h
                            for qc in range(2):
                                nc.tensor.matmul(
                                    ps_av[h][qc][:, :],
                                    vhat[kt // 4][:, kt % 4, head, :],
                                    es[h][:, qc * 512:(qc + 1) * 512],
                                    start=(kt == 0), stop=(kt == 15),
                                    skip_group_check=True)
                    # normalize: gather the 4 sumexp rows (PSUM row 64) into
                    # 32-aligned partitions of se, one reciprocal over all 4;
                    # sel rows broadcast them across partitions via matmul.
                    # All PSUM->SBUF evacuations are emitted first (avs2 on
                    # ACT, se rows on DVE) so the av banks free up quickly
                    # for the next block's accumulation.
                    se = tmp.tile([128, 512], F32, tag="se", name=f"se{pair}{qh}")
                    nc.vector.memset(se, 1.0)
                    avs2 = [tmp.tile([128, 512], F32, tag=f"avs2{qc}",
                                     name=f"avs{pair}{qh}{qc}")
                            for qc in range(2)]
                    for h in range(2):
                        for qc in range(2):
                            nc.scalar.activation(avs2[qc][h * 64:(h + 1) * 64, :],
                                                 ps_av[h][qc][0:64, :], AF.Copy)
                            r0 = 32 * (2 * h + qc)
                            nc.vector.tensor_copy(se[r0:r0 + 1, :],
                                                  ps_av[h][qc][64:65, :])
                    nc.vector._custom_dve(RECIPROCAL_APPROX_FAST,
                                          out=recip4[:, :], in0=se[:, :],
                                          s0=_c["s0"], s1=_c["s1"],
                                          imm2=_c["imm2"])
                    for qc in range(2):
                        pb2 = ps2.tile([128, 512], F32, tag=f"av0{qc}",
                                       name=f"nb{pair}{qh}{qc}")
                        nc.tensor.matmul(pb2[:, :], sel[:, qc, :], recip4[:, :],
                                         start=True, stop=True)
                        nc.vector.tensor_mul(
                            vmix[pair][qh][:, qc * 512:(qc + 1) * 512],
                            avs2[qc][:, :], pb2[:, :])

        # ---------------- phase 3: out proj ----------------
        with tc.tile_pool(name="ps3", bufs=1, space="PSUM") as ps3:
            heat3 = ps3.tile([128, 512], F32, tag="po0", bufs=2, name="heat3")
            for i in range(10):
                nc.tensor.matmul(heat3[:, :], Pm[:, :], qt[0][0][:, :],
                                 start=True, stop=True, skip_group_check=True)
            for st in range(16):
                po = [ps3.tile([128, 512], F32, tag=f"po{n}", bufs=2,
                               name=f"po{st}_{n}") for n in range(2)]
                for t in range(2):     # n inner: LDWEIGHTS shared across n
                    for n in range(2):
                        nc.tensor.matmul(
                            po[n][:, :],
                            vmix[t][st // 8][:, (st % 8) * 128:(st % 8 + 1) * 128],
                            wout[:, t, n * 512:(n + 1) * 512],
                            start=(t == 0), stop=(t == 1),
                            skip_group_check=True)
                for n in range(2):
                    o = outp.tile([128, 512], BF16, tag="o", name=f"o{st}_{n}")
                    if (st * 2 + n) % 2 == 0:
                        nc.scalar.activation(o[:, :], po[n][:, :], AF.Copy)
                    else:
                        nc.vector.tensor_copy(o[:, :], po[n][:, :])
                    nc.sync.dma_start(
                        out=out_d.ap()[st * 128:(st + 1) * 128,
                                       n * 512:(n + 1) * 512],
                        in_=o[:, :])

    nc.compile()
    return nc


def host_prep(x, pos, Wqkv, bqkv, Wout, bout, q_scale, k_scale):
    """Build per-core input maps + shared-table decision."""
    x = np.asarray(x, dtype=np.float32)
    pos = np.asarray(pos, dtype=np.float32).reshape(-1)
    Wqkv = np.asarray(Wqkv, dtype=np.float32)
    bqkv = np.asarray(bqkv, dtype=np.float32)
    Wout = np.asarray(Wout, dtype=np.float32)
    q_scale = np.asarray(q_scale, dtype=np.float32)
    k_scale = np.asarray(k_scale, dtype=np.float32)

    shared = bool(np.array_equal(q_scale, k_scale))
    exp_scale = (1.0 / np.sqrt(DH)) if shared else 1.0

    bf = ml_dtypes.bfloat16

    # rope base tables [128, S]
    i_of_p = (np.arange(128) % 64) // 2            # pair index
    sign = np.where(np.arange(128) % 2 == 0, 1.0, -1.0)
    omega = THETA ** (-np.arange(0, DH, 2, dtype=np.float64) / DH)  # [32]
    ang = pos[None, :].astype(np.float64) * omega[:, None]          # [32, S]
    cosb = np.cos(ang)[i_of_p, :]                  # [128, S]
    sinb = np.sin(ang)[i_of_p, :] * sign[:, None]

    def tables(scale_vec, extra):
        sv = np.tile(scale_vec, 2)                 # [128]
        svx = np.tile(scale_vec[np.arange(64) ^ 1], 2)
        cosT = (cosb * sv[:, None] * extra).astype(bf)
        sinT = (sinb * svx[:, None] * extra).astype(bf)
        return np.ascontiguousarray(cosT), np.ascontiguousarray(sinT)

    cos_k, sin_k = tables(k_scale, 1.0)
    if not shared:
        cos_q, sin_q = tables(q_scale, 1.0 / np.sqrt(DH))

    Pm = np.zeros((128, 128), dtype=np.float32)
    Pm[np.arange(128), np.arange(128) ^ 1] = 1.0
    Pm = Pm.astype(bf)
    onesblk = np.zeros((128, 2), dtype=np.float32)
    onesblk[0:64, 0] = 1.0
    onesblk[64:128, 1] = 1.0
    onesblk = onesblk.astype(bf)
    ones2blk = np.zeros((2, 128), dtype=np.float32)
    ones2blk[0, 0:64] = 1.0
    ones2blk[1, 64:128] = 1.0
    # sel[qc]: [4, 128] selecting reciprocal row (h, qc) for partitions h*64..
    sel = np.zeros((128, 2, 128), dtype=np.float32)
    for qc in range(2):
        for h in range(2):
            sel[32 * (2 * h + qc), qc, h * 64:(h + 1) * 64] = 1.0

    in_maps = []
    for c in range(NC):
        b, g = c // 4, c % 4
        xT = np.ascontiguousarray(
            x[b].T.reshape(NDT, 128, S).transpose(1, 0, 2)).astype(bf)
        wq = Wqkv[:, g * HD:(g + 1) * HD]
        wk = Wqkv[:, DM + g * HD: DM + (g + 1) * HD]
        wv = Wqkv[:, 2 * DM + g * HD: 2 * DM + (g + 1) * HD]
        w_all = np.ascontiguousarray(
            np.concatenate([wq, wk, wv], axis=1)
            .reshape(NDT, 128, 3 * HD).transpose(1, 0, 2)).astype(bf)
        wo = np.ascontiguousarray(
            Wout[g * HD:(g + 1) * HD, :]
            .reshape(2, 128, DM).transpose(1, 0, 2)).astype(bf)
        bqs = np.ascontiguousarray(
            bqkv[g * HD:(g + 1) * HD].reshape(2, 128).T)         # [128, 2]
        bks = np.ascontiguousarray(
            bqkv[DM + g * HD: DM + (g + 1) * HD].reshape(2, 128).T)
        m = {"xT": xT, "w_all": w_all, "wout": wo, "bq": bqs, "bk": bks,
             "cos_k": cos_k, "sin_k": sin_k, "Pswap": Pm, "onesblk": onesblk,
             "ones2blk": ones2blk, "sel": sel}
        if not shared:
            m["cos_q"] = cos_q
            m["sin_q"] = sin_q
        in_maps.append(m)

    bias_row = (bqkv[2 * DM:] @ Wout + np.asarray(bout, dtype=np.float32)) \
        .astype(np.float32)                                       # [1024]
    return in_maps, shared, float(exp_scale), bias_row


def _install_ntff_shim():
    """Make trace=True usable: this image lacks antenv.axon_hooks; recreate
    it against the baked libaxon_pjrt.so C ABI (no-op if already present)."""
    try:
        from antenv.axon_hooks import get_axon_ntff_profile_hook  # noqa: F401
        return
    except ImportError:
        pass
    try:
        import types, ctypes, contextlib
        import antenv
        lib = ctypes.CDLL("/opt/axon/libaxon_pjrt.so")
        if not hasattr(lib, "axon_start_nrt_profile"):
            raise OSError("no profile symbols")
        lib.axon_start_nrt_profile.argtypes = [ctypes.POINTER(ctypes.c_int64),
                                               ctypes.c_size_t]
        lib.axon_start_nrt_profile.restype = ctypes.c_int64
        lib.axon_stop_nrt_profile.argtypes = [ctypes.c_char_p]
        lib.axon_stop_nrt_profile.restype = ctypes.c_int64

        @contextlib.contextmanager
        def _hook(output_dir, device_ids):
            import jax
            jax.devices()
            if device_ids:
                ids = (ctypes.c_int64 * len(device_ids))(*device_ids)
                rc = lib.axon_start_nrt_profile(ids, len(device_ids))
            else:
                rc = lib.axon_start_nrt_profile(None, 0)
            if rc != 0:
                raise RuntimeError(f"axon_start_nrt_profile rc={rc}")
            try:
                yield
            finally:
                lib.axon_stop_nrt_profile(str(output_dir).encode())

        mod = types.ModuleType("antenv.axon_hooks")
        mod.get_axon_ntff_profile_hook = lambda: _hook
        mod.set_axon_ntff_profile_hook = lambda h: None
        sys.modules["antenv.axon_hooks"] = mod
        antenv.axon_hooks = mod
    except Exception:
        os.environ["BASS_NEVER_TRACE"] = "1"   # degrade: run untraced


def kernel(x, pos, Wqkv, bqkv, Wout, bout, q_scale, k_scale):
    global LAST_RESULTS
    if os.environ.get("BASS_TRACE"):
        _install_ntff_shim()
    in_maps, shared, exp_scale, bias_row = host_prep(
        x, pos, Wqkv, bqkv, Wout, bout, q_scale, k_scale)

    key = (shared, round(exp_scale, 9))
    if key not in _CACHED:
        _CACHED[key] = build_program(exp_scale, shared)
    nc = _CACHED[key]

    res = bass_utils.run_bass_kernel_spmd(
        nc, in_maps, list(range(NC)),
        trace=bool(os.environ.get("BASS_TRACE")))
    LAST_RESULTS = res

    out = np.empty((B, S, DM), dtype=np.float32)
    for b in range(B):
        acc = bias_row[None, :].astype(np.float32).repeat(S, axis=0)
        for g in range(4):
            acc = acc + res.results[b * 4 + g]["outp"].astype(np.float32)
        out[b] = acc
    return out


# revision 24
# speedup vs baseline: 1.1839x; 1.1839x over previous
"""Trainium2 Bass kernel for nn_Attention_32650341384246.

Full attention layer: qkv proj + per-head RMSNorm(q,k) + RoPE + softmax
attention (non-causal) + out proj.  B=2, S=2048, D=1024, H=16, DH=64.

Sharding: 8 cores; core c handles batch c//4, heads [4*(c%4), 4*(c%4)+4)
(data parallel over batch x tensor parallel over heads).  Each core
computes a partial [S, D] output (its heads @ Wout row-slice); the host
sums the 4 partials per batch and adds the (folded) biases.

Device design (per core):
  - x fed pre-transposed+bf16 as xT [128, 8, 2048]  (p + 128*a = model dim)
  - qkv proj emits qT/kT head-major [128 (2 heads x 64), S] directly
    (lhsT = W slice, rhs = xT slice) and v s-major [s, 4*64].
  - RMSNorm in head-major layout: sum(x^2) over d via ones-block matmul,
    rsqrt directly on ACT (AF.Abs_reciprocal_sqrt; its table set also has
    identity -> exactly 2 ACT table loads in the whole kernel: one in
    phase 1, the exp set in phase 2), partition-broadcast via ones matmul.
  - RoPE as q_rot = cosT*u + sinT'*swap(u); swap = adjacent-partition
    permutation matmul; cos/sin tables host-built bf16 from `pos` with
    q_scale/k_scale folded in; rope DVE muls run bf16 (2x mode where
    both operands are SBUF bf16).
  - scores^T [k, q] bf16 matmuls (K=64, tile_position row groups, qc
    inner so each LDWEIGHTS serves 2 matmuls), PSUM [128, 1024] per
    head.
  - exp split across engines to unblock the PE: head 0 of each pair on
    ACT (table exp, bf16 out), head 1 on DVE via a Schraudolph fast-exp
    (bits16 = round(score*c + d) -> int16 tile bitcast bf16; the
    constant bias of the approximation cancels exactly in softmax).
  - AV via lhsT = [v | ones] bf16 (M=65): row 64 accumulates sumexp.
  - normalize: per-row RECIPROCAL_APPROX_FAST straight off the PSUM
    sumexp rows into 32-aligned partitions of one table, select-matrix
    matmul broadcasts reciprocal rows across partitions.
  - out proj: lhsT = v_mixT bf16, rhs = Wout row-slice bf16, n inner so
    each LDWEIGHTS serves 2 matmuls.
"""
import sys, os

sys.path.insert(0, "/opt/trn_rl_repo")

import numpy as np
from contextlib import ExitStack

import ml_dtypes
import concourse.bass as bass
import concourse.mybir as mybir
import concourse.tile as tile
from concourse import bacc
from concourse import bass_utils

F32 = mybir.dt.float32
F32R = mybir.dt.float32r
BF16 = mybir.dt.bfloat16
I16 = mybir.dt.int16
AF = mybir.ActivationFunctionType
ALU = mybir.AluOpType

B, S, DM, H, DH = 2, 2048, 1024, 16, 64
NC = 8
HPC = H // 4          # 4 heads per core
HD = HPC * DH         # 256
NDT = DM // 128       # 8 model-dim tiles
THETA, EPS = 10000.0, 1e-6
LOG2E = 1.4426950408889634

# kt iterations (of 16) whose pair-head-1 exp runs on DVE (Schraudolph);
# the rest go to ACT.  Tuned so ACT and DVE finish together.
DVE_KTS = frozenset(range(16))

LAST_RESULTS = None   # BassKernelResults of the most recent device run
_CACHED = {}


def build_program(exp_scale: float, shared_tables: bool):
    nc = bacc.Bacc("TRN2", target_bir_lowering=False, debug=False)

    xT_d = nc.dram_tensor("xT", [128, NDT, S], BF16, kind="ExternalInput")
    w_d = nc.dram_tensor("w_all", [128, NDT, 3 * HD], BF16, kind="ExternalInput")
    wout_d = nc.dram_tensor("wout", [128, 2, DM], BF16, kind="ExternalInput")
    bq_d = nc.dram_tensor("bq", [128, 2], F32, kind="ExternalInput")
    bk_d = nc.dram_tensor("bk", [128, 2], F32, kind="ExternalInput")
    cosk_d = nc.dram_tensor("cos_k", [128, S], BF16, kind="ExternalInput")
    sink_d = nc.dram_tensor("sin_k", [128, S], BF16, kind="ExternalInput")
    if not shared_tables:
        cosq_d = nc.dram_tensor("cos_q", [128, S], BF16, kind="ExternalInput")
        sinq_d = nc.dram_tensor("sin_q", [128, S], BF16, kind="ExternalInput")
    P_d = nc.dram_tensor("Pswap", [128, 128], BF16, kind="ExternalInput")
    ob_d = nc.dram_tensor("onesblk", [128, 2], BF16, kind="ExternalInput")
    o2_d = nc.dram_tensor("ones2blk", [2, 128], F32R, kind="ExternalInput")
    sel_d = nc.dram_tensor("sel", [128, 2, 128], F32R, kind="ExternalInput")
    out_d = nc.dram_tensor("outp", [S, DM], BF16, kind="ExternalOutput")

    # Schraudolph fast-exp constants: bits16(x) = x*sc_c + sc_d, int16,
    # bitcast bf16 ~= 2^(x*exp_scale*log2e + const).  The 2^const factor
    # cancels in softmax normalization.
    sc_c = float(exp_scale * 128.0 * LOG2E)
    sc_d = float(127.0 * 128.0 - 6.0)

    with tile.TileContext(nc) as tc, ExitStack() as ctx, \
            nc.allow_low_precision(reason="fp32r/bf16 matmul inputs"):
        singles = ctx.enter_context(tc.tile_pool(name="singles", bufs=1))
        tmp = ctx.enter_context(tc.tile_pool(name="tmp", bufs=2))
        expp = ctx.enter_context(tc.tile_pool(name="expp", bufs=4))
        outp = ctx.enter_context(tc.tile_pool(name="outp", bufs=4))

        # --- first-needed loads up front; per-dt tiles so Tile's
        # per-tile RAW tracking doesn't serialize readers behind all DMAs ---
        w_dt = [singles.tile([128, 3 * HD], BF16, name=f"w{dt}") for dt in range(NDT)]
        x_dt = [singles.tile([128, S], BF16, name=f"x{dt}") for dt in range(NDT)]
        for dt in range(NDT):
            nc.sync.dma_start(out=w_dt[dt], in_=w_d.ap()[:, dt, :])
            nc.sync.dma_start(out=x_dt[dt], in_=xT_d.ap()[:, dt, :])

        wout = singles.tile([128, 2, DM], BF16)
        nc.sync.dma_start(out=wout, in_=wout_d.ap())
        bq = singles.tile([128, 2], F32)
        nc.sync.dma_start(out=bq, in_=bq_d.ap())
        bk = singles.tile([128, 2], F32)
        nc.sync.dma_start(out=bk, in_=bk_d.ap())
        cos_k = singles.tile([128, S], BF16)
        nc.sync.dma_start(out=cos_k, in_=cosk_d.ap())
        sin_k = singles.tile([128, S], BF16)
        nc.sync.dma_start(out=sin_k, in_=sink_d.ap())
        if shared_tables:
            cos_q, sin_q = cos_k, sin_k
        else:
            cos_q = singles.tile([128, S], BF16)
            nc.sync.dma_start(out=cos_q, in_=cosq_d.ap())
            sin_q = singles.tile([128, S], BF16)
            nc.sync.dma_start(out=sin_q, in_=sinq_d.ap())
        Pm = singles.tile([128, 128], BF16)
        nc.sync.dma_start(out=Pm, in_=P_d.ap())
        onesblk = singles.tile([128, 2], BF16)
        nc.sync.dma_start(out=onesblk, in_=ob_d.ap())
        ones2blk = singles.tile([2, 128], F32R)
        nc.sync.dma_start(out=ones2blk, in_=o2_d.ap())
        sel = singles.tile([128, 2, 128], F32R)
        nc.sync.dma_start(out=sel, in_=sel_d.ap())
        eps_t = singles.tile([128, 1], F32)
        nc.vector.memset(eps_t, EPS)
        recip4 = singles.tile([128, 512], F32R)

        qt = [[singles.tile([128, 512], BF16, name=f"qt{t}_{sc}")
               for sc in range(4)] for t in range(2)]
        kt_ = [[singles.tile([128, 512], BF16, name=f"kt{t}_{sc}")
                for sc in range(4)] for t in range(2)]
        # v columns padded to 128 (cols 65..127 zero) so the AV LDWEIGHTS
        # sees a full 128-column bf16 weight and fast-weight-load kicks in
        vhat = [singles.tile([128, 4, HPC, 128], BF16, name=f"vhat{sc}")
                for sc in range(4)]
        for sc in range(4):
            nc.vector.memset(vhat[sc][:, :, :, 64:128], 0.0)
            nc.vector.memset(vhat[sc][:, :, :, 64:65], 1.0)
        vmix = [[singles.tile([128, 1024], BF16, name=f"vmix{t}_{qh}")
                 for qh in range(2)] for t in range(2)]

        # ---------------- phase 1: qkv + rmsnorm + rope ----------------
        # section-major order [k0, q0, k1, q1, v].  ACT uses only
        # Identity + Rsqrt here (one table set); rope DVE muls are bf16.
        with tc.tile_pool(name="ps1", bufs=1, space="PSUM") as ps1:
            sections = (
                    ("k", 0, bk, cos_k, sin_k, kt_),
                    ("q", 0, bq, cos_q, sin_q, qt),
                    ("k", 1, bk, cos_k, sin_k, kt_),
                    ("q", 1, bq, cos_q, sin_q, qt),
                    ("v", -1, None, None, None, None))
            for which, t, bias, cosT, sinT, dest in sections:
                if which == "v":
                    for sc in range(4):
                        for st in range(4):
                            pv = ps1.tile([128, HD], F32, tag="pv", bufs=2,
                                          name=f"pv{sc}_{st}")
                            for dt in range(NDT):
                                nc.tensor.matmul(
                                    pv[:, :],
                                    x_dt[dt][:, sc * 512 + st * 128: sc * 512 + (st + 1) * 128],
                                    w_dt[dt][:, 2 * HD:3 * HD],
                                    start=(dt == 0), stop=(dt == NDT - 1))
                            nc.vector.tensor_copy(vhat[sc][:, st, :, 0:64],
                                                  pv[:, :].rearrange("p (h d) -> p h d", h=HPC))
                    continue
                off = 0 if which == "q" else HD

                def tail(which, t, bias, cosT, sinT, dest, sc, pq):
                    s0 = sc * 512
                    tt = tmp.tile([128, 512], BF16, tag="tt", bufs=5,
                                  name=f"tt{which}{t}_{sc}")
                    nc.scalar.activation(tt[:, :], pq[:, :], AF.Identity,
                                         bias=bias[:, t:t + 1], scale=1.0)
                    sq = tmp.tile([128, 512], BF16, tag="sq", name=f"sq{which}{t}_{sc}")
                    nc.vector.tensor_mul(sq[:, :], tt[:, :], tt[:, :])
                    pss = ps1.tile([2, 512], F32, tag="pss", bufs=2,
                                   name=f"pss{which}{t}_{sc}")
                    nc.tensor.matmul(pss[:, :], onesblk[:, :], sq[:, :],
                                     start=True, stop=True)
                    rs = tmp.tile([2, 512], F32R, tag="rs", name=f"rs{which}{t}_{sc}")
                    nc.scalar.activation(rs[:, :], pss[:, :],
                                         AF.Abs_reciprocal_sqrt,
                                         bias=eps_t[0:2, :], scale=1.0 / DH)
                    pb = ps1.tile([128, 512], F32, tag="pb",
                                  name=f"pb{which}{t}_{sc}")
                    nc.tensor.matmul(pb[:, :], ones2blk[:, :], rs[:, :],
                                     start=True, stop=True)
                    u = tmp.tile([128, 512], BF16, tag="u", name=f"u{which}{t}_{sc}")
                    nc.vector.tensor_mul(u[:, :], tt[:, :], pb[:, :])
                    psw = ps1.tile([128, 512], F32, tag="psw",
                                   name=f"psw{which}{t}_{sc}")
                    nc.tensor.matmul(psw[:, :], Pm[:, :], u[:, :],
                                     start=True, stop=True)
                    t1 = tmp.tile([128, 512], BF16, tag="t1", name=f"t1{which}{t}_{sc}")
                    nc.vector.tensor_mul(t1[:, :], u[:, :], cosT[:, s0:s0 + 512])
                    t2 = tmp.tile([128, 512], BF16, tag="t2", name=f"t2{which}{t}_{sc}")
                    nc.vector.tensor_mul(t2[:, :], psw[:, :], sinT[:, s0:s0 + 512])
                    nc.vector.tensor_add(dest[t][sc][:, :], t1[:, :], t2[:, :])

                for scp in range(2):      # s-chunk pairs: dt-outer inside a
                    # pair so consecutive matmuls share one LDWEIGHTS
                    pqs = [ps1.tile([128, 512], F32, tag="pq", bufs=2,
                                    name=f"pq{which}{t}_{scp * 2 + j}")
                           for j in range(2)]
                    for dt in range(NDT):
                        for j in range(2):
                            nc.tensor.matmul(
                                pqs[j][:, :],
                                w_dt[dt][:, off + t * 128: off + (t + 1) * 128],
                                x_dt[dt][:, (scp * 2 + j) * 512:(scp * 2 + j + 1) * 512],
                                start=(dt == 0), stop=(dt == NDT - 1),
                                skip_group_check=True)
                    for j in range(2):
                        tail(which, t, bias, cosT, sinT, dest, scp * 2 + j,
                             pqs[j])

        # ---------------- phase 2: attention ----------------
        from concourse.dve_ops import (RECIP_APPROX_FAST_CONSTS,
                                       RECIPROCAL_APPROX_FAST)
        _c = RECIP_APPROX_FAST_CONSTS
        with tc.tile_pool(name="ps2", bufs=1, space="PSUM") as ps2:
            # HAM heater: ~3.5us of gapless back-to-back matmuls so the PE
            # clock-gate opens (1.2 -> 2.4 GHz) before the attention loop,
            # whose exp-paced micro-gaps never present a full busy window.
            heat = ps2.tile([128, 512], F32, tag="av00", name="heat")
            for i in range(16):
                nc.tensor.matmul(heat[:, :], Pm[:, :], qt[0][0][:, :],
                                 start=True, stop=True, skip_group_check=True)
            for pair in range(2):
                for qh in range(2):
                    q0 = qh * 1024
                    ps_sc = [ps2.tile([128, 1024], F32, tag=f"sc{h}",
                                      name=f"sc{pair}{qh}{h}") for h in range(2)]
                    ps_av = [[ps2.tile([128, 512], F32, tag=f"av{h}{qc}",
                                       name=f"av{pair}{qh}{h}{qc}")
                              for qc in range(2)] for h in range(2)]
                    for kt in range(16):
                        # scores: h outer (LDWEIGHTS shared across the two
                        # qc matmuls)
                        for h in range(2):
                            for qc in range(2):
                                nc.tensor.matmul(
                                    ps_sc[h][:, qc * 512:(qc + 1) * 512],
                                    kt_[pair][kt // 4][h * 64:(h + 1) * 64,
                                                       (kt % 4) * 128:(kt % 4 + 1) * 128],
                                    qt[pair][qh * 2 + qc][h * 64:(h + 1) * 64, :],
                                    start=True, stop=True,
                                    tile_position=(h * 64, 0))
                        # exp: h0 on ACT (table), h1 on DVE (Schraudolph)
                        e0 = expp.tile([128, 1024], BF16, tag="e0",
                                       name=f"e{pair}{qh}0_{kt}")
                        nc.scalar.activation(e0[:, :], ps_sc[0][:, :], AF.Exp,
                                             scale=exp_scale)
                        e1i = expp.tile([128, 1024], I16, tag="e1",
                                        name=f"e{pair}{qh}1_{kt}")
                        nc.vector.tensor_scalar(
                            e1i[:, :], ps_sc[1][:, :], sc_c, sc_d,
                            ALU.mult, ALU.add)
                        e1 = e1i.bitcast(BF16)
                        es = (e0, e1)
                        for h in range(2):
                            head = 2 * pair + h
                            for qc in range(2):
                                nc.tensor.matmul(
                                    ps_av[h][qc][:, :],
                                    vhat[kt // 4][:, kt % 4, head, :],
                                    es[h][:, qc * 512:(qc + 1) * 512],
                                    start=(kt == 0), stop=(kt == 15),
                                    skip_group_check=True)
                    # normalize: gather the 4 sumexp rows (PSUM row 64) into
                    # 32-aligned partitions of se, one reciprocal over all 4;
                    # sel rows broadcast them across partitions via matmul.
                    # All PSUM->SBUF evacuations are emitted first (avs2 on
                    # ACT, se rows on DVE) so the av banks free up quickly
                    # for the next block's accumulation.
                    se = tmp.tile([128, 512], F32, tag="se", name=f"se{pair}{qh}")
                    nc.vector.memset(se, 1.0)
                    avs2 = [tmp.tile([128, 512], F32, tag=f"avs2{qc}",
                                     name=f"avs{pair}{qh}{qc}")
                            for qc in range(2)]
                    for h in range(2):
                        for qc in range(2):
                            nc.scalar.activation(avs2[qc][h * 64:(h + 1) * 64, :],
                                                 ps_av[h][qc][0:64, :], AF.Copy)
                            r0 = 32 * (2 * h + qc)
                            nc.vector.tensor_copy(se[r0:r0 + 1, :],
                                                  ps_av[h][qc][64:65, :])
                    nc.vector._custom_dve(RECIPROCAL_APPROX_FAST,
                                          out=recip4[:, :], in0=se[:, :],
                                          s0=_c["s0"], s1=_c["s1"],
                                          imm2=_c["imm2"])
                    for qc in range(2):
                        pb2 = ps2.tile([128, 512], F32, tag=f"av0{qc}",
                                       name=f"nb{pair}{qh}{qc}")
                        nc.tensor.matmul(pb2[:, :], sel[:, qc, :], recip4[:, :],
                                         start=True, stop=True)
                        nc.vector.tensor_mul(
                            vmix[pair][qh][:, qc * 512:(qc + 1) * 512],
                            avs2[qc][:, :], pb2[:, :])

        # ---------------- phase 3: out proj ----------------
        with tc.tile_pool(name="ps3", bufs=1, space="PSUM") as ps3:
            heat3 = ps3.tile([128, 512], F32, tag="po0", bufs=2, name="heat3")
            for i in range(10):
                nc.tensor.matmul(heat3[:, :], Pm[:, :], qt[0][0][:, :],
                                 start=True, stop=True, skip_group_check=True)
            for st in range(16):
                po = [ps3.tile([128, 512], F32, tag=f"po{n}", bufs=2,
                               name=f"po{st}_{n}") for n in range(2)]
                for t in range(2):     # n inner: LDWEIGHTS shared across n
                    for n in range(2):
                        nc.tensor.matmul(
                            po[n][:, :],
                            vmix[t][st // 8][:, (st % 8) * 128:(st % 8 + 1) * 128],
                            wout[:, t, n * 512:(n + 1) * 512],
                            start=(t == 0), stop=(t == 1),
                            skip_group_check=True)
                for n in range(2):
                    o = outp.tile([128, 512], BF16, tag="o", name=f"o{st}_{n}")
                    if (st * 2 + n) % 2 == 0:
                        nc.scalar.activation(o[:, :], po[n][:, :], AF.Copy)
                    else:
                        nc.vector.tensor_copy(o[:, :], po[n][:, :])
                    nc.sync.dma_start(
                        out=out_d.ap()[st * 128:(st + 1) * 128,
                                       n * 512:(n + 1) * 512],
                        in_=o[:, :])

    nc.compile()
    return nc


def host_prep(x, pos, Wqkv, bqkv, Wout, bout, q_scale, k_scale):
    """Build per-core input maps + shared-table decision."""
    x = np.asarray(x, dtype=np.float32)
    pos = np.asarray(pos, dtype=np.float32).reshape(-1)
    Wqkv = np.asarray(Wqkv, dtype=np.float32)
    bqkv = np.asarray(bqkv, dtype=np.float32)
    Wout = np.asarray(Wout, dtype=np.float32)
    q_scale = np.asarray(q_scale, dtype=np.float32)
    k_scale = np.asarray(k_scale, dtype=np.float32)

    shared = bool(np.array_equal(q_scale, k_scale))
    exp_scale = (1.0 / np.sqrt(DH)) if shared else 1.0

    bf = ml_dtypes.bfloat16

    # rope base tables [128, S]
    i_of_p = (np.arange(128) % 64) // 2            # pair index
    sign = np.where(np.arange(128) % 2 == 0, 1.0, -1.0)
    omega = THETA ** (-np.arange(0, DH, 2, dtype=np.float64) / DH)  # [32]
    ang = pos[None, :].astype(np.float64) * omega[:, None]          # [32, S]
    cosb = np.cos(ang)[i_of_p, :]                  # [128, S]
    sinb = np.sin(ang)[i_of_p, :] * sign[:, None]

    def tables(scale_vec, extra):
        sv = np.tile(scale_vec, 2)                 # [128]
        svx = np.tile(scale_vec[np.arange(64) ^ 1], 2)
        cosT = (cosb * sv[:, None] * extra).astype(bf)
        sinT = (sinb * svx[:, None] * extra).astype(bf)
        return np.ascontiguousarray(cosT), np.ascontiguousarray(sinT)

    cos_k, sin_k = tables(k_scale, 1.0)
    if not shared:
        cos_q, sin_q = tables(q_scale, 1.0 / np.sqrt(DH))

    Pm = np.zeros((128, 128), dtype=np.float32)
    Pm[np.arange(128), np.arange(128) ^ 1] = 1.0
    Pm = Pm.astype(bf)
    onesblk = np.zeros((128, 2), dtype=np.float32)
    onesblk[0:64, 0] = 1.0
    onesblk[64:128, 1] = 1.0
    onesblk = onesblk.astype(bf)
    ones2blk = np.zeros((2, 128), dtype=np.float32)
    ones2blk[0, 0:64] = 1.0
    ones2blk[1, 64:128] = 1.0
    # sel[qc]: [4, 128] selecting reciprocal row (h, qc) for partitions h*64..
    sel = np.zeros((128, 2, 128), dtype=np.float32)
    for qc in range(2):
        for h in range(2):
            sel[32 * (2 * h + qc), qc, h * 64:(h + 1) * 64] = 1.0

    in_maps = []
    for c in range(NC):
        b, g = c // 4, c % 4
        xT = np.ascontiguousarray(
            x[b].T.reshape(NDT, 128, S).transpose(1, 0, 2)).astype(bf)
        wq = Wqkv[:, g * HD:(g + 1) * HD]
        wk = Wqkv[:, DM + g * HD: DM + (g + 1) * HD]
        wv = Wqkv[:, 2 * DM + g * HD: 2 * DM + (g + 1) * HD]
        w_all = np.ascontiguousarray(
            np.concatenate([wq, wk, wv], axis=1)
            .reshape(NDT, 128, 3 * HD).transpose(1, 0, 2)).astype(bf)
        wo = np.ascontiguousarray(
            Wout[g * HD:(g + 1) * HD, :]
            .reshape(2, 128, DM).transpose(1, 0, 2)).astype(bf)
        bqs = np.ascontiguousarray(
            bqkv[g * HD:(g + 1) * HD].reshape(2, 128).T)         # [128, 2]
        bks = np.ascontiguousarray(
            bqkv[DM + g * HD: DM + (g + 1) * HD].reshape(2, 128).T)
        m = {"xT": xT, "w_all": w_all, "wout": wo, "bq": bqs, "bk": bks,
             "cos_k": cos_k, "sin_k": sin_k, "Pswap": Pm, "onesblk": onesblk,
             "ones2blk": ones2blk, "sel": sel}
        if not shared:
            m["cos_q"] = cos_q
            m["sin_q"] = sin_q
        in_maps.append(m)

    bias_row = (bqkv[2 * DM:] @ Wout + np.asarray(bout, dtype=np.float32)) \
        .astype(np.float32)                                       # [1024]
    return in_maps, shared, float(exp_scale), bias_row


def _install_ntff_shim():
    """Make trace=True usable: this image lacks antenv.axon_hooks; recreate
    it against the baked libaxon_pjrt.so C ABI (no-op if already present)."""
    try:
        from antenv.axon_hooks import get_axon_ntff_profile_hook  # noqa: F401
        return
    except ImportError:
        pass
    try:
        import types, ctypes, contextlib
        import antenv
        lib = ctypes.CDLL("/opt/axon/libaxon_pjrt.so")
        if not hasattr(lib, "axon_start_nrt_profile"):
            raise OSError("no profile symbols")
        lib.axon_start_nrt_profile.argtypes = [ctypes.POINTER(ctypes.c_int64),
                                               ctypes.c_size_t]
        lib.axon_start_nrt_profile.restype = ctypes.c_int64
        lib.axon_stop_nrt_profile.argtypes = [ctypes.c_char_p]
        lib.axon_stop_nrt_profile.restype = ctypes.c_int64

        @contextlib.contextmanager
        def _hook(output_dir, device_ids):
            import jax
            jax.devices()
            if device_ids:
                ids = (ctypes.c_int64 * len(device_ids))(*device_ids)
                rc = lib.axon_start_nrt_profile(ids, len(device_ids))
            else:
                rc = lib.axon_start_nrt_profile(None, 0)
            if rc != 0:
                raise RuntimeError(f"axon_start_nrt_profile rc={rc}")
            try:
                yield
            finally:
                lib.axon_stop_nrt_profile(str(output_dir).encode())

        mod = types.ModuleType("antenv.axon_hooks")
        mod.get_axon_ntff_profile_hook = lambda: _hook
        mod.set_axon_ntff_profile_hook = lambda h: None
        sys.modules["antenv.axon_hooks"] = mod
        antenv.axon_hooks = mod
    except Exception:
        os.environ["BASS_NEVER_TRACE"] = "1"   # degrade: run untraced


def kernel(x, pos, Wqkv, bqkv, Wout, bout, q_scale, k_scale):
    global LAST_RESULTS
    if os.environ.get("BASS_TRACE"):
        _install_ntff_shim()
    in_maps, shared, exp_scale, bias_row = host_prep(
        x, pos, Wqkv, bqkv, Wout, bout, q_scale, k_scale)

    key = (shared, round(exp_scale, 9))
    if key not in _CACHED:
        _CACHED[key] = build_program(exp_scale, shared)
    nc = _CACHED[key]

    res = bass_utils.run_bass_kernel_spmd(
        nc, in_maps, list(range(NC)),
        trace=bool(os.environ.get("BASS_TRACE")))
    LAST_RESULTS = res

    out = np.empty((B, S, DM), dtype=np.float32)
    for b in range(B):
        acc = bias_row[None, :].astype(np.float32).repeat(S, axis=0)
        for g in range(4):
            acc = acc + res.results[b * 4 + g]["outp"].astype(np.float32)
        out[b] = acc
    return out
